# revision 24
# baseline (speedup 1.0000x reference)
"""Trainium2 Bass kernel for Enformer-style relative-position attention.

Problem: nn_Attention_79087527788690
  x [1, 2048, 1536] -> out [1, 2048, 1536]
  8 heads, dk=64, dv=192, rel-pos features=192, n=2048.

Sharding: one head per NeuronCore (8 cores). Each core computes its head's
q/k/v projections, content + relative-position logits, softmax, attention
output and a partial out-projection [2048, 1536]; chunked ReduceScatters
(one per 512-row group, pipelined with compute) sum the partials across
cores; the host reassembles the row strips.

The relative_shift is realized with a DRAM round trip: for each q-tile the
rel-logit window G[p, u] (u in a 2175-wide span starting at s0 = 1920 - i0)
is written contiguously to DRAM ([128, 2176] pitch) and read back with a
skewed access pattern (row stride 2175, offset 127), which gives
shifted[p, j] = G[p, 127 - p + j] without cross-partition traffic.
"""

import math
import sys
from contextlib import ExitStack

sys.path.insert(0, "/opt/trn_rl_repo")

import numpy as np

N = 2048
DIM = 1536
HEADS = 8
DK = 64
DV = 192
F = 192  # rel pos features
SPAN = 2 * N - 1  # 4095
NCORES = 8
CHUNK = N // NCORES  # 256
SCALE = DK ** -0.5

IT = 128          # q rows per tile
NIT = N // IT     # 16
JC = 512          # j chunk for logits
NJC = N // JC     # 4
GW = N + IT - 1   # 2175, G window per i-tile
GPITCH = 2176     # padded pitch of the DRAM G buffer
NGRP = 4          # reduce-scatter groups
GRP = N // NGRP   # 512 rows per group
STRIP = GRP // NCORES  # 64 rows per (group, core)


def _positions() -> np.ndarray:
    """get_positional_embed(2048, 192) in numpy (f64 -> f32). [4095, 192]"""
    d = np.arange(-N + 1, N).astype(np.float64)
    nb = F // 6
    absd = np.abs(d)[:, None]
    max_range = math.log(N) / math.log(2.0)
    half_life = 2.0 ** np.linspace(3.0, max_range, nb)
    feat_exp = np.exp(-math.log(2.0) / half_life[None, :] * absd)
    cw = 2.0 ** np.arange(1, nb + 1) - 1.0
    feat_cm = (cw[None, :] > absd).astype(np.float64)
    stddev = N / (2 * nb)
    start_mean = N / nb
    mean = np.linspace(start_mean, N, nb)[None, :]
    conc = (mean / stddev) ** 2
    rate = mean / stddev ** 2
    with np.errstate(divide="ignore", invalid="ignore"):
        log_unnorm = (conc - 1.0) * np.log(absd) - rate * absd
    log_unnorm = np.where(absd == 0, -np.inf, log_unnorm)
    lg = np.vectorize(math.lgamma)(conc)
    log_norm = lg - conc * np.log(rate)
    probs = np.exp(log_unnorm - log_norm) + 1e-8
    feat_gamma = probs / np.amax(probs, axis=-1, keepdims=True)
    emb = np.concatenate([feat_exp, feat_cm, feat_gamma], axis=-1)
    out = np.concatenate([emb, np.sign(d)[:, None] * emb], axis=-1)
    return out.astype(np.float32)


def build_nc(num_cores: int = NCORES, collective: bool = True):
    """Build + compile the per-core Bass graph (SPMD, identical on all cores)."""
    import concourse.bass as bass
    import concourse.mybir as mybir
    import concourse.tile as tile
    from concourse import bacc
    from concourse.masks import make_identity

    f32 = mybir.dt.float32
    f16 = mybir.dt.float16
    bf16 = mybir.dt.bfloat16

    nc = bacc.Bacc(
        "TRN2", target_bir_lowering=False, debug=False, num_devices=num_cores
    )

    # --- external I/O (per-core shards supplied via in_maps) ---
    xT_e = nc.dram_tensor("xT", [DIM, N], f16, kind="ExternalInput")
    wq_e = nc.dram_tensor("wq", [DIM, DK], f16, kind="ExternalInput")  # SCALE folded
    wk_e = nc.dram_tensor("wk", [DIM, DK], f16, kind="ExternalInput")
    wv_e = nc.dram_tensor("wv", [DIM, DV], f16, kind="ExternalInput")
    wrel_e = nc.dram_tensor("wrel", [F, DK], f16, kind="ExternalInput")
    posT_e = nc.dram_tensor("posT", [F, SPAN], f16, kind="ExternalInput")
    rcb_e = nc.dram_tensor("rcb", [DK, 1], f32, kind="ExternalInput")
    rpb_e = nc.dram_tensor("rpb", [DK, 1], f32, kind="ExternalInput")
    wo_e = nc.dram_tensor("wo", [DV, DIM], f16, kind="ExternalInput")
    bo_e = nc.dram_tensor("bo", [1, DIM], f32, kind="ExternalInput")
    out_rows = CHUNK if collective else N
    out_e = nc.dram_tensor("out", [out_rows, DIM], f32, kind="ExternalOutput")

    # --- internal DRAM ---
    cc_ins = [nc.dram_tensor(f"cc_in{g}", [GRP, DIM], bf16) for g in range(NGRP)]
    cc_outs = [nc.dram_tensor(f"cc_out{g}", [STRIP, DIM], bf16) for g in range(NGRP)]
    NGD = 3
    gds = [nc.dram_tensor(f"gd{i}", [IT, GPITCH], f16) for i in range(NGD)]

    with tile.TileContext(nc) as tc, ExitStack() as ctx:
        const = ctx.enter_context(tc.tile_pool(name="const", bufs=1))
        work = ctx.enter_context(tc.tile_pool(name="work", bufs=2))
        psum = ctx.enter_context(tc.tile_pool(name="psum", bufs=2, space="PSUM"))

        # ---- constants / weights into SBUF ----
        ident_b = const.tile([128, 128], bf16, tag="idb")
        make_identity(nc, ident_b[:])
        ident_h = const.tile([128, 128], f16, tag="idh")
        make_identity(nc, ident_h[:])

        xT = const.tile([128, 12, N], f16, tag="xT")  # d-chunk c at [:, c, :]
        wqk_s = const.tile([128, 12, 2 * DK], f16, tag="wqk")
        wv_s = const.tile([128, 12, DV], f16, tag="wv")
        for c in range(12):
            nc.sync.dma_start(out=wqk_s[:, c, 0:DK],
                              in_=wq_e[128 * c:128 * (c + 1), :])
            nc.sync.dma_start(out=wqk_s[:, c, DK:2 * DK],
                              in_=wk_e[128 * c:128 * (c + 1), :])
            nc.sync.dma_start(out=wv_s[:, c, :], in_=wv_e[128 * c:128 * (c + 1), :])
        wrel0 = const.tile([128, DK], f16, tag="wrel0")
        wrel1 = const.tile([64, DK], f16, tag="wrel1")
        nc.sync.dma_start(out=wrel0[:], in_=wrel_e[0:128, :])
        nc.sync.dma_start(out=wrel1[:], in_=wrel_e[128:192, :])
        pos0 = const.tile([128, SPAN], f16, tag="pos0")
        pos1 = const.tile([64, SPAN], f16, tag="pos1")
        nc.sync.dma_start(out=pos0[:], in_=posT_e[0:128, :])
        nc.sync.dma_start(out=pos1[:], in_=posT_e[128:192, :])
        rcb_s = const.tile([DK, 1], f32, tag="rcb")
        rpb_s = const.tile([DK, 1], f32, tag="rpb")
        nc.sync.dma_start(out=rcb_s[:], in_=rcb_e[:])
        nc.sync.dma_start(out=rpb_s[:], in_=rpb_e[:])
        wo0 = const.tile([96, DIM], f16, tag="wo0")
        wo1 = const.tile([96, DIM], f16, tag="wo1")
        nc.sync.dma_start(out=wo0[:], in_=wo_e[0:96, :])
        nc.sync.dma_start(out=wo1[:], in_=wo_e[96:192, :])
        bo_s = const.tile([1, DIM], f32, tag="bo")
        nc.sync.dma_start(out=bo_s[:], in_=bo_e[:])
        bo128 = const.tile([128, DIM], f32, tag="bo128")
        nc.gpsimd.partition_broadcast(bo128[:], bo_s[:])
        # x last: the small weights above unblock the first matmuls sooner
        for c in range(12):
            nc.sync.dma_start(out=xT[:, c, :], in_=xT_e[128 * c:128 * (c + 1), :])

        # rel_k^T [64, 4095(+1 pad)] f16
        relkT = const.tile([DK, SPAN + 1], f16, tag="relkT")
        for sc in range(8):
            w = min(512, SPAN - 512 * sc)
            pr = psum.tile([DK, 512], f32, tag="mm")
            nc.tensor.matmul(pr[:, 0:w], wrel0[:], pos0[:, 512 * sc:512 * sc + w],
                             start=True, stop=False)
            nc.tensor.matmul(pr[:, 0:w], wrel1[:], pos1[:, 512 * sc:512 * sc + w],
                             start=False, stop=True)
            nc.vector.tensor_copy(relkT[:, 512 * sc:512 * sc + w], pr[:, 0:w])

        # ---- projections ----
        qcT = const.tile([DK, N], f16, tag="qcT")  # (q*s + rcb)^T
        qpT = const.tile([DK, N], f16, tag="qpT")  # (q*s + rpb)^T
        kT = const.tile([DK, N], f16, tag="kT")
        vb = const.tile([128, NIT, DV], bf16, tag="vb")  # j-tile jt at [:, jt, :]

        for ic in range(N // 512):
            pq = psum.tile([128, 512], f32, tag="pc")
            for c in range(12):
                nc.tensor.matmul(pq[:], wqk_s[:, c, :],
                                 xT[:, c, 512 * ic:512 * (ic + 1)],
                                 start=(c == 0), stop=(c == 11))
            nc.scalar.activation(qcT[:, 512 * ic:512 * (ic + 1)], pq[0:DK, :],
                                 mybir.ActivationFunctionType.Identity,
                                 bias=rcb_s[:], scale=1.0)
            nc.scalar.activation(qpT[:, 512 * ic:512 * (ic + 1)], pq[0:DK, :],
                                 mybir.ActivationFunctionType.Identity,
                                 bias=rpb_s[:], scale=1.0)
            nc.vector.tensor_copy(kT[:, 512 * ic:512 * (ic + 1)], pq[DK:2 * DK, :])

        for jt in range(NIT):
            pv = psum.tile([128, DV], f32, tag="mm")
            for c in range(12):
                nc.tensor.matmul(pv[:], xT[:, c, IT * jt:IT * (jt + 1)],
                                 wv_s[:, c, :], start=(c == 0), stop=(c == 11))
            nc.vector.tensor_copy(vb[:, jt, :], pv[:])

        # ---- main attention loop (G stage software-pipelined 1 tile ahead) ----
        def emit_g(it):
            """Rel-logit window matmuls + DRAM shift round trip for tile it."""
            i0 = IT * it
            w0 = (N - IT) - i0  # window start s0 = 1920 - i0
            gwin = work.tile([128, GPITCH], f16, tag="gwin", name=f"gwin{it}")
            for q in range(4):
                pg = psum.tile([128, JC], f32, tag="mm", name=f"pg{it}_{q}")
                nc.tensor.matmul(pg[:], qpT[:, i0:i0 + IT],
                                 relkT[:, w0 + JC * q:w0 + JC * (q + 1)],
                                 start=True, stop=True)
                nc.vector.tensor_copy(gwin[:, JC * q:JC * (q + 1)], pg[:])
            pg2 = psum.tile([128, IT], f32, tag="g2", bufs=1, name=f"pg2_{it}")
            nc.tensor.matmul(pg2[:, 0:IT - 1], qpT[:, i0:i0 + IT],
                             relkT[:, w0 + 4 * JC:w0 + GW], start=True, stop=True)
            nc.vector.tensor_copy(gwin[:, 4 * JC:GW], pg2[:, 0:IT - 1])
            gd = gds[it % NGD]
            nc.sync.dma_start(out=gd[:, 0:GW], in_=gwin[:, 0:GW])
            diag = bass.AP(gd, 127, [[GW, 128], [1, N]])
            rel = work.tile([128, N], f16, tag="rel", name=f"rel{it}")
            nc.sync.dma_start(out=rel[:], in_=diag)
            return rel

        def emit_logits(it, rel):
            """Content logits + shifted rel + exp for tile it -> (E, rcp)."""
            i0 = IT * it
            E = work.tile([128, N], bf16, tag="E", name=f"E{it}")
            rs4 = work.tile([128, NJC], f32, tag="rs4", name=f"rs4_{it}")
            for jc in range(NJC):
                j0 = JC * jc
                pc = psum.tile([128, JC], f32, tag="pc", name=f"pc{it}_{jc}")
                nc.tensor.matmul(pc[:], qcT[:, i0:i0 + IT], kT[:, j0:j0 + JC],
                                 start=True, stop=False)
                # accumulate the shifted rel logits via identity matmul
                nc.tensor.matmul(pc[:], ident_h[:], rel[:, j0:j0 + JC],
                                 start=False, stop=True)
                nc.scalar.activation(E[:, j0:j0 + JC], pc[:],
                                     mybir.ActivationFunctionType.Exp,
                                     accum_out=rs4[:, jc:jc + 1])
            rs = work.tile([128, 1], f32, tag="rs", name=f"rs{it}")
            nc.vector.reduce_sum(rs[:], rs4[:], axis=mybir.AxisListType.X)
            rcp = work.tile([128, 1], f32, tag="rcp", name=f"rcp{it}")
            nc.vector.reciprocal(rcp[:], rs[:])
            return E, rcp

        def emit_pv(it, E, rcp):
            """PV + out-projection partial (+ group reduce-scatter) for tile it."""
            i0 = IT * it
            # PV: accumulate over j tiles with transposed E blocks.
            # 4 transposes share one PSUM tile so one copy moves 4 blocks.
            po = psum.tile([128, DV], f32, tag="o", bufs=1, name=f"po{it}")
            for jq in range(NIT // 4):
                pt4 = psum.tile([128, 4, 128], bf16, tag="tr4", name=f"pt4_{it}_{jq}")
                for q in range(4):
                    jt = 4 * jq + q
                    nc.tensor.transpose(pt4[:, q, :], E[:, IT * jt:IT * (jt + 1)],
                                        ident_b[:])
                etb4 = work.tile([128, 4, 128], bf16, tag="etb4",
                                 name=f"etb4_{it}_{jq}")
                if jq % 2 == 0:
                    nc.vector.tensor_copy(etb4[:], pt4[:])
                else:
                    nc.scalar.copy(etb4[:], pt4[:])
                for q in range(4):
                    jt = 4 * jq + q
                    nc.tensor.matmul(po[:], etb4[:, q, :], vb[:, jt, :],
                                     start=(jt == 0), stop=(jt == NIT - 1),
                                     skip_group_check=True)
            oh = work.tile([128, DV], f16, tag="oh", name=f"oh{it}")
            nc.vector.tensor_scalar(oh[:], po[:], rcp[:], None,
                                    mybir.AluOpType.mult)
            # transpose oh -> ohT (c-chunks of 96)
            ohT = work.tile([96, 2, 128], f16, tag="ohT", name=f"ohT{it}")
            for h in range(2):
                pth = psum.tile([96, 128], f16, tag="g2", bufs=1,
                                name=f"pth{it}_{h}")
                nc.tensor.transpose(pth[:], oh[:, 96 * h:96 * (h + 1)], ident_h[:])
                nc.vector.tensor_copy(ohT[:, h, :], pth[:])
            # partial out-projection [128, 1536] for this i-tile
            g = it // (NIT // NGRP)
            r0 = i0 - g * GRP  # row offset within the group
            for ec in range(3):
                pp = psum.tile([128, JC], f32, tag="mm", name=f"pp{it}_{ec}")
                nc.tensor.matmul(pp[:], ohT[:, 0, :], wo0[:, JC * ec:JC * (ec + 1)],
                                 start=True, stop=False)
                nc.tensor.matmul(pp[:], ohT[:, 1, :], wo1[:, JC * ec:JC * (ec + 1)],
                                 start=False, stop=True)
                part = work.tile([128, JC], bf16 if collective else f32,
                                 tag="part", bufs=3, name=f"part{it}_{ec}")
                if ec % 2 == 0:
                    nc.vector.tensor_copy(part[:], pp[:])
                else:
                    nc.scalar.copy(part[:], pp[:])
                if collective:
                    nc.sync.dma_start(
                        out=cc_ins[g][r0:r0 + IT, JC * ec:JC * (ec + 1)],
                        in_=part[:])
                else:
                    nc.sync.dma_start(
                        out=out_e[i0:i0 + IT, JC * ec:JC * (ec + 1)], in_=part[:])
            # chunked reduce-scatter once a row group is complete
            if collective and it % (NIT // NGRP) == NIT // NGRP - 1:
                nc.gpsimd.collective_compute(
                    "ReduceScatter",
                    mybir.AluOpType.add,
                    replica_groups=[list(range(num_cores))],
                    ins=[cc_ins[g][:]],
                    outs=[cc_outs[g][:]],
                )
                res = work.tile([STRIP, DIM], bf16, tag="res", name=f"res{g}")
                nc.sync.dma_start(out=res[:], in_=cc_outs[g][:])
                fin = work.tile([STRIP, DIM], f32, tag="fin", name=f"fin{g}")
                nc.vector.tensor_tensor(fin[:], res[:], bo128[0:STRIP, :],
                                        mybir.AluOpType.add)
                nc.sync.dma_start(out=out_e[STRIP * g:STRIP * (g + 1), :],
                                  in_=fin[:])

        # drive: G leads by one tile, PV trails exp by one tile
        rel_next = emit_g(0)
        prev = None
        for it in range(NIT):
            rel = rel_next
            if it + 1 < NIT:
                rel_next = emit_g(it + 1)
            cur = (it, emit_logits(it, rel))
            if prev is not None:
                emit_pv(prev[0], *prev[1])
            prev = cur
        emit_pv(prev[0], *prev[1])

    nc.compile()
    return nc


_CACHE: dict = {}


def _get_nc():
    if "nc" not in _CACHE:
        _CACHE["nc"] = build_nc()
    return _CACHE["nc"]


def _shard_inputs(x, Wq, Wk, Wv, Wrel, rel_content_bias, rel_pos_bias, Wo, bo):
    posT = np.ascontiguousarray(_positions().T).astype(np.float16)  # [192, 4095]
    xT = np.ascontiguousarray(
        np.asarray(x, np.float32).reshape(N, DIM).T).astype(np.float16)
    in_maps = []
    for h in range(NCORES):
        in_maps.append({
            "xT": xT,
            "wq": np.ascontiguousarray(
                Wq[:, DK * h:DK * (h + 1)] * SCALE).astype(np.float16),
            "wk": np.ascontiguousarray(
                Wk[:, DK * h:DK * (h + 1)]).astype(np.float16),
            "wv": np.ascontiguousarray(
                Wv[:, DV * h:DV * (h + 1)]).astype(np.float16),
            "wrel": np.ascontiguousarray(
                Wrel[:, DK * h:DK * (h + 1)]).astype(np.float16),
            "posT": posT,
            "rcb": np.ascontiguousarray(
                rel_content_bias[0, h, 0, :].reshape(DK, 1)).astype(np.float32),
            "rpb": np.ascontiguousarray(
                rel_pos_bias[0, h, 0, :].reshape(DK, 1)).astype(np.float32),
            "wo": np.ascontiguousarray(
                Wo[DV * h:DV * (h + 1), :]).astype(np.float16),
            "bo": np.asarray(bo, np.float32).reshape(1, DIM),
        })
    return in_maps


def kernel(**inputs) -> np.ndarray:
    from concourse.bass_utils import run_bass_kernel_spmd

    inputs = {k: np.asarray(v) for k, v in inputs.items()}
    nc = _get_nc()
    in_maps = _shard_inputs(**inputs)
    res = run_bass_kernel_spmd(nc, in_maps, list(range(NCORES)))
    # core h's out rows: strip g -> global rows GRP*g + STRIP*h + [0, STRIP)
    out = np.empty((N, DIM), np.float32)
    for h in range(NCORES):
        oc = np.asarray(res.results[h]["out"])
        for g in range(NGRP):
            out[GRP * g + STRIP * h:GRP * g + STRIP * (h + 1), :] = \
                oc[STRIP * g:STRIP * (g + 1), :]
    return out.reshape(1, N, DIM)
